# revision 1
# baseline (speedup 1.0000x reference)
"""BitLinear (absmean ternary quantized linear) on 8 TRN2 NeuronCores.

out[b,t,o] = sum_i x[b,t,i] * (clip(round(W[o,i]/delta), -1, 1) * delta) + bias[o]
delta = mean(|W|) + 1e-8  over the FULL weight.

Sharding: tensor-parallel over OUT rows (11008 / 8 = 1376 rows per core).
x is replicated. delta partial abs-sums are AllGathered across the 8 cores.
Host passes each core its weight shard transposed ([IN, OUT_SH], contiguous)
so the contraction dim lands on SBUF partitions; host concatenates the 8
output shards.

Quantization without round() (not available on any engine):
  2q = 2*1[w >= d/2] - 2*1[w <= -d/2]                         (a.e.)
     = sign(w - d/2) + sign(w + d/2)                          (a.e.)
The matmul distributes over the two threshold maps (exact in bf16, both
scaled to 2q units), each feeding its own matmul stream; the epilogue
applies out = (delta/2) * psum with bias folded in via a K=1 PSUM-init
matmul of bias*(2/delta).

W is held as PAIR tiles (2 k-tiles per SBUF tile): halves the DMA trigger,
reduce, map-op and semaphore counts (ScalarE pays ~0.3us fixed cost per
instruction). Map pairs are split between ACT (sign) and DVE (is_ge/is_le)
lanes to balance engine time. PE is kept warm across the collective gap
with a paced ping-pong chain plus a dense bf16 burst gated on the
thresholds, so the real matmuls start at full clock.
"""

import numpy as np

B, T, IN, OUT = 8, 16, 4096, 11008
M = B * T               # 128 tokens
CORES = 8
OUT_SH = OUT // CORES   # 1376
KT = IN // 128          # 32 k-tiles
NP = KT // 2            # 16 pair-tiles
N_TOTAL_W = OUT * IN    # 45088768
EPS = 1e-8

RES_PAIRS = 11          # pair tiles kept SBUF-resident between pass A and B
STR_PAIRS = NP - RES_PAIRS  # first STR_PAIRS pairs stream through wstream
WS_BUFS = 2             # wstream pair slots
A_PAIRS = 5             # pairs quantized on ACT (sign); rest on DVE
COL_SLICES = [(0, 512), (512, 1024), (1024, OUT_SH)]
GAP_CHAIN = 22          # PE<->DVE ping-pong links bridging the collective gap
WARM_BURST = 48         # dense N=256 bf16 matmuls to flip HAM warm pre pass B


def _lane_assignment():
    quotas = {"A": A_PAIRS, "D": NP - A_PAIRS}
    used = {k: 0 for k in quotas}
    lanes = []
    for p in range(NP):
        lane = max(quotas, key=lambda l: quotas[l] * (p + 1) / NP - used[l])
        used[lane] += 1
        lanes.append(lane)
    return lanes


LANES = _lane_assignment()

_CACHE = {}


def _build():
    from concourse import bass, bacc, tile, mybir

    f32 = mybir.dt.float32
    bf16 = mybir.dt.bfloat16
    AF = mybir.ActivationFunctionType
    ALU = mybir.AluOpType

    nc = bacc.Bacc("TRN2", target_bir_lowering=False, debug=False, num_devices=CORES)

    wt_d = nc.dram_tensor("wt", [IN, OUT_SH], f32, kind="ExternalInput")
    xt_d = nc.dram_tensor("xt", [IN, M], f32, kind="ExternalInput")
    bias_d = nc.dram_tensor("bias", [1, OUT_SH], f32, kind="ExternalInput")
    out_d = nc.dram_tensor("out", [M, OUT_SH], f32, kind="ExternalOutput")

    def pair_dma(eng, dst_ap, p):
        r0 = 256 * p
        eng.dma_start(
            out=dst_ap,
            in_=wt_d[r0 : r0 + 256, :].rearrange("(two q) c -> q two c", q=128),
        )

    with tile.TileContext(nc) as tc:
        with (
            tc.tile_pool(name="wres", bufs=RES_PAIRS) as wres,
            tc.tile_pool(name="wstream", bufs=WS_BUFS) as wstream,
            tc.tile_pool(name="xstage", bufs=2) as xstage,
            tc.tile_pool(name="xp", bufs=1) as xp,
            tc.tile_pool(name="bp", bufs=1) as bp,
            tc.tile_pool(name="cons", bufs=1) as cons,
            tc.tile_pool(name="stat", bufs=1) as stat,
            tc.tile_pool(name="maps", bufs=3) as maps,
            tc.tile_pool(name="op", bufs=1) as op,
            tc.tile_pool(name="dram", bufs=1, space="DRAM") as dram,
            tc.tile_pool(name="psmall", bufs=1, space="PSUM") as psmall,
            tc.tile_pool(name="pjunk", bufs=1, space="PSUM") as pjunk,
            tc.tile_pool(name="pout", bufs=1, space="PSUM") as pout,
        ):
            # ---- weight DMAs first: they are the memory roofline ----
            w_pairs = {}
            for p in range(NP):
                if p >= STR_PAIRS:
                    wp = wres.tile([128, 2, OUT_SH], f32, tag="w")
                else:
                    wp = wstream.tile([128, 2, OUT_SH], f32, tag="ws")
                pair_dma(nc.sync, wp[:], p)
                w_pairs[p] = wp
                if p == 2:
                    bias_sb = bp.tile([1, OUT_SH], f32)
                    nc.sync.dma_start(out=bias_sb[:], in_=bias_d[:])

            # ---- constants / small tiles ----
            ones_col = cons.tile([128, 1], f32)
            ones_row = cons.tile([1, 128], f32)
            nc.gpsimd.memset(ones_col[:], 1.0)
            nc.gpsimd.memset(ones_row[:], 1.0)
            ones2d = cons.tile([128, 128], f32)
            nc.gpsimd.memset(ones2d[:], 1.0)
            ones_row_bf = cons.tile([1, 128], bf16)
            nc.gpsimd.memset(ones_row_bf[:], 1.0)
            jrow_bf = cons.tile([1, 256], bf16)
            warm = cons.tile([128, 1], f32)
            # pre-load the ACT table set containing Sign while DMAs run
            nc.scalar.activation(warm[:], ones_col[:], AF.Sign)

            partials = stat.tile([128, NP], f32)
            sumP = stat.tile([128, 1], f32)
            s_sb = stat.tile([1, 8], f32)
            gath = stat.tile([8, 8], f32)
            d_sb = stat.tile([1, 1], f32)
            rd2_sb = stat.tile([1, 1], f32)
            dh_bc = stat.tile([128, 1], f32)    # delta/2 broadcast (epilogue)
            th = stat.tile([128, 1], f32)       # +delta/2
            nth = stat.tile([128, 1], f32)      # -delta/2
            junk_sb = stat.tile([128, 1], f32)
            wjunk = stat.tile([8, 8], f32)

            # early dummy collective: absorbs the cold ncfw cost during
            # pass A so AG1/AG2 run on the warm path
            ccw_in = dram.tile([1, 8], f32)
            ccw_out = dram.tile([8, 8], f32, addr_space="Shared")
            nc.gpsimd.dma_start(out=ccw_in[:], in_=ones_row[0:1, 0:8])
            nc.gpsimd.collective_compute(
                "AllGather",
                ALU.bypass,
                replica_groups=[list(range(CORES))],
                ins=[ccw_in[:].opt()],
                outs=[ccw_out[:].opt()],
            )
            nc.gpsimd.dma_start(out=wjunk[:], in_=ccw_out[:])

            psum_out = pout.tile([M, OUT_SH], f32)
            junk_ps = pjunk.tile([128, 512], f32)

            # ---- pass A: abs-sum each weight PAIR as it lands ----
            for p in range(NP):
                nc.vector.tensor_reduce(
                    partials[:, p : p + 1],
                    w_pairs[p][:],
                    axis=mybir.AxisListType.XY,
                    op=ALU.add,
                    apply_absolute_value=True,
                )

            # ---- delta: local sum -> AllGather (runs on the warm path
            # behind the early dummy collective) ----
            nc.vector.tensor_reduce(
                sumP[:], partials[:], axis=mybir.AxisListType.X, op=ALU.add
            )
            ps1 = psmall.tile([1, 1], f32, tag="ps1")
            nc.tensor.matmul(ps1[:], sumP[:], ones_col[:])  # sum over partitions
            nc.gpsimd.memset(s_sb[:], 0.0)
            nc.vector.tensor_copy(s_sb[0:1, 0:1], ps1[:])

            cc_in = dram.tile([1, 8], f32)
            cc_out = dram.tile([8, 8], f32, addr_space="Shared")
            nc.gpsimd.dma_start(out=cc_in[:], in_=s_sb[:])
            nc.gpsimd.collective_compute(
                "AllGather",
                ALU.bypass,
                replica_groups=[list(range(CORES))],
                ins=[cc_in[:].opt()],
                outs=[cc_out[:].opt()],
            )
            nc.gpsimd.dma_start(out=gath[:], in_=cc_out[:])

            # ---- x: 4 staged strided DMAs (scalar queue) + bf16 casts on
            # DVE right after the reduces; ready by ~ the collective's end ----
            xbf = xp.tile([128, KT, M], bf16)   # x.T in bf16 (all lanes)
            for c in range(4):
                xs = xstage.tile([128, 8, M], f32, tag="xs")
                nc.sync.dma_start(
                    out=xs[:],
                    in_=xt_d[1024 * c : 1024 * (c + 1), :].rearrange(
                        "(t q) c -> q t c", q=128
                    ),
                )
                for t in range(8):
                    nc.vector.tensor_copy(xbf[:, 8 * c + t, :], xs[:, t, :])

            # streamed pairs' pass-B re-DMAs: issued here so they fire in the
            # collective gap (their wstream slot frees after pass-A reduces)
            for p in range(STR_PAIRS):
                wp = wstream.tile([128, 2, OUT_SH], f32, tag="ws")
                pair_dma(nc.sync, wp[:], p)
                w_pairs[p] = wp

            # PE warm-keeper chain across the collective gap: PE <-> DVE
            # ping-pong; each link's latency spaces the matmuls out in time.
            nc.tensor.matmul(junk_ps[:, 0:1], ones_row[:], sumP[0:1, 0:1])
            for _ in range(GAP_CHAIN):
                nc.vector.tensor_copy(junk_sb[:], junk_ps[:, 0:1])
                nc.tensor.matmul(junk_ps[:, 0:1], ones_row[:], junk_sb[0:1, 0:1])

            # S summed over cores AND broadcast to 128 partitions in ONE mm:
            # ones2d[0:8,:].T @ gath[0:8,0:1] -> [128,1] of S_total
            psb = psmall.tile([128, 1], f32, tag="psb")
            nc.tensor.matmul(psb[:], ones2d[0:8, :], gath[0:8, 0:1])
            # thresholds straight from PSUM: th = S*(0.5/N) + eps/2 = delta/2
            nc.vector.tensor_scalar(
                th[:], psb[:], 0.5 / N_TOTAL_W, EPS / 2, op0=ALU.mult, op1=ALU.add
            )
            nc.vector.tensor_scalar(
                nth[:], psb[:], -0.5 / N_TOTAL_W, -EPS / 2, op0=ALU.mult, op1=ALU.add
            )
            # off-critical-path: epilogue scale delta/2 and bias scale 2/delta
            nc.vector.tensor_copy(dh_bc[:], th[:])
            nc.vector.tensor_scalar(
                d_sb[:], psb[0:1, 0:1], 0.5 / N_TOTAL_W, EPS / 2,
                op0=ALU.mult, op1=ALU.add,
            )
            nc.vector.reciprocal(rd2_sb[:], d_sb[:])  # 2/delta

            # dense warm burst gated on th: flips HAM to 8/8 in the ~3.5us
            # right before the real matmuls start (maps overlap the burst)
            nc.vector.tensor_scalar(
                jrow_bf[0:1, 0:1], th[0:1, 0:1], 1.0, None, op0=ALU.mult
            )
            nc.gpsimd.memset(jrow_bf[0:1, 1:256], 1.0)
            for _ in range(WARM_BURST):
                nc.tensor.matmul(junk_ps[:, 0:256], ones_row_bf[:], jrow_bf[:])

            # bias*(2/delta) into PSUM via K=1 ones matmul (broadcast rows)
            nc.vector.tensor_scalar(
                bias_sb[:], bias_sb[:], rd2_sb[:], None, op0=ALU.mult
            )
            for c0, c1 in COL_SLICES:
                nc.tensor.matmul(
                    psum_out[:, c0:c1], ones_row[:], bias_sb[:, c0:c1],
                    start=True, stop=False,
                )

            # ---- pass B: quantize + matmul per PAIR; streamed pairs early
            # then spread so their slots recycle during the gap ----
            pass_b_order = [p for p in range(NP) if p >= STR_PAIRS]
            for i, p in enumerate(range(STR_PAIRS)):
                pass_b_order.insert(1 + 3 * i, p)
            assert sorted(pass_b_order) == list(range(NP))
            for pi, p in enumerate(pass_b_order):
                wp = w_pairs[p]
                mA = maps.tile([128, 2, OUT_SH], bf16, tag="mA")
                mB = maps.tile([128, 2, OUT_SH], bf16, tag="mB")
                if LANES[p] == "A":
                    # sign method on ACT over the whole pair (one op each)
                    nc.scalar.activation(mA[:], wp[:], AF.Sign, bias=nth[:])
                    nc.scalar.activation(mB[:], wp[:], AF.Sign, bias=th[:])
                else:
                    # threshold method on DVE: 2q = 2a - 2b, folded into maps
                    nc.vector.tensor_scalar(
                        mA[:], wp[:], th[:], 2.0, op0=ALU.is_ge, op1=ALU.mult
                    )
                    nc.vector.tensor_scalar(
                        mB[:], wp[:], nth[:], -2.0, op0=ALU.is_le, op1=ALU.mult
                    )
                last = pi == NP - 1
                for j in range(2):
                    xa = xbf[:, 2 * p + j, :]
                    for c0, c1 in COL_SLICES:
                        nc.tensor.matmul(
                            psum_out[:, c0:c1], xa, mA[:, j, c0:c1],
                            start=False, stop=False,
                        )
                    for si, (c0, c1) in enumerate(COL_SLICES):
                        nc.tensor.matmul(
                            psum_out[:, c0:c1], xa, mB[:, j, c0:c1],
                            start=False, stop=last and j == 1 and si == 2,
                        )

            # epilogue: out = (delta/2) * psum  (bias already in, pre-scaled)
            out_sb = op.tile([M, OUT_SH], f32)
            for c0, c1 in COL_SLICES:
                nc.vector.tensor_scalar(
                    out_sb[:, c0:c1], psum_out[:, c0:c1], dh_bc[:], None,
                    op0=ALU.mult,
                )
                nc.sync.dma_start(out=out_d[:, c0:c1], in_=out_sb[:, c0:c1])

    nc.compile()
    return nc


def _get_nc():
    if "nc" not in _CACHE:
        _CACHE["nc"] = _build()
    return _CACHE["nc"]


def _run(x, weight, bias, **spmd_kwargs):
    from concourse.bass_utils import run_bass_kernel_spmd

    x = np.ascontiguousarray(np.asarray(x), dtype=np.float32)
    weight = np.ascontiguousarray(np.asarray(weight), dtype=np.float32)
    bias = np.ascontiguousarray(np.asarray(bias), dtype=np.float32)

    xt = np.ascontiguousarray(x.reshape(M, IN).T)  # [IN, M]
    in_maps = []
    for c in range(CORES):
        rows = slice(c * OUT_SH, (c + 1) * OUT_SH)
        in_maps.append(
            {
                "xt": xt,
                "wt": np.ascontiguousarray(weight[rows].T),  # [IN, OUT_SH]
                "bias": bias[rows].reshape(1, OUT_SH),
            }
        )
    nc = _get_nc()
    res = run_bass_kernel_spmd(nc, in_maps, core_ids=list(range(CORES)), **spmd_kwargs)
    out = np.concatenate([res.results[c]["out"] for c in range(CORES)], axis=1)
    return out.reshape(B, T, OUT).astype(np.float32), res


def kernel(x, weight, bias):
    out, _ = _run(x, weight, bias)
    return out

